# revision 14
# baseline (speedup 1.0000x reference)
"""Trainium2 Bass kernel for nn_CombinedModel_80315888435653.

Pipeline (per forward):
  1. per-element FFN emulators  ->  summed spectrum y      [N=50125]
  2. banded Gaussian velocity broadening (halfwidth 300)   [N]
  3. redshift + linear rebin onto instrument grid          [N_NEW=20000]
  4. ARF scale + response matmul rm @ folded               [N_CHAN=4096]

Distribution strategy (8 NeuronCores, SPMD, full inputs in / full out):
  * Channel-sharded, fully independent cores - NO collective.  Core c
    owns instrument channels [2500c, 2500c+2500); it computes the summed
    spectrum y only on the 6016-bin energy window its channels touch
    (incl. the +-300 broadening halo), broadens it, interpolates, and
    multiplies its 2500-column slice of the response matrix.  Host sums
    the 8 partial [4096] outputs.  Windows of neighbouring cores overlap
    by ~600 bins (~10% duplicated Wout traffic) - far cheaper than an
    AllGather + replicated broadening.
  * The dominant HBM stream (Wout, 27 MB/core) and the response slice
    (10.5 MB/core) are shipped as fp8e4 (scales folded in, x64 exponent
    trick compensated in the exp bias) and consumed with DoubleRow
    matmuls (2 fp8 K-chunks per instruction).
  * Broadening is 18 accumulated Toeplitz matmuls in bf16 (the Gaussian
    collapses to a 601-tap shift-invariant kernel on the log grid).
  * Interp is a per-core static block-sparse fp8 matrix; everything is
    scale-normalized on host so fp8 dynamic range is safe, and the one
    global scalar is applied to the final host-side sum.
"""
import math
import os
import sys
from contextlib import ExitStack

import numpy as np

for _p in ('/opt/trn_rl_repo', '/root/.axon_site/_ro/trn_rl_repo'):
    if os.path.isdir(_p) and _p not in sys.path:
        sys.path.insert(0, _p)

import ml_dtypes  # noqa: E402
import concourse.bass as bass  # noqa: E402
import concourse.tile as tile  # noqa: E402
from concourse import bacc, mybir  # noqa: E402
from concourse.bass_utils import run_bass_kernel_spmd  # noqa: E402

bf16 = ml_dtypes.bfloat16
f8e4 = ml_dtypes.float8_e4m3
f32 = np.float32

# ---- problem constants ----
C_LIGHT = 299792458.0
N = 50125
BAND = 300
E = 30
HID = 150
NNEW = 20000
NCHAN = 4096
LD = 3.086e24

# ---- plan constants ----
NCORES = 8
KR = E * HID                 # 4500 contraction rows
NPAIR = 18                   # 36 K-chunks as 18 DoubleRow pairs
KPAD = NPAIR * 256           # 4608
E2 = 32                      # element rows padded to 32 (DoubleRow stride)
NWIN = 6016                  # per-core einsum window (47 * 128)
WCOLS = NWIN // 128          # 47
SUP = [1024, 1024, 1024, 1024, 1024, 896]   # supercol widths (sum NWIN)
VSH = 384                    # 3-col left halo shift in v
TCOLS = 48                   # transpose-padded col count (%16)
LPAD = 16                    # left pad cols (16-aligned XBAR transpose dst)
VCOLS = LPAD + TCOLS + 9     # 73
UOFF = LPAD - 3              # conv rhs col offset (A_pm embeds VSH=384=3 cols)
C0 = 2                       # S window starts at broad col 2 (bin off 256)
NBLK = 20                    # fold blocks (128 channels each)
SCH = 4                      # S K-chunks per block
TCH = NNEW // NCORES         # 2500 channels per core
TCAP = NBLK * 128            # 2560
LN10 = float(np.log(10.0))
WSCALE = 64.0                # fp8 Wout prescale, compensated in exp bias

RHO = (math.log10(50.0) - math.log10(0.15)) / (NNEW - 1) / (3.0 / (N - 1))
CB = [max(0, math.floor((128 * b * RHO - 4) / 128.0)) for b in range(NBLK)]


# ----------------------------------------------------------------------
# device program (built & compiled once per process)
# ----------------------------------------------------------------------
_NC = None


def _build_nc(sim_single=False, stop_after=None, debug_taps=False):
    dt = mybir.dt
    DR = mybir.MatmulPerfMode.DoubleRow
    nc = bacc.Bacc("TRN2", target_bir_lowering=False, debug=False,
                   num_devices=1 if sim_single else NCORES)

    wout = nc.dram_tensor("wout", [KPAD, NWIN], dt.float8e4, kind="ExternalInput").ap()
    hbd = nc.dram_tensor("hbd", [128, NPAIR * 2 * E2], dt.float8e4, kind="ExternalInput").ap()
    means = nc.dram_tensor("means", [E2, NWIN], dt.bfloat16, kind="ExternalInput").ap()
    a_pm = nc.dram_tensor("a_pm", [128, 9 * 128], dt.bfloat16, kind="ExternalInput").ap()
    dxw = nc.dram_tensor("dxw", [128, VCOLS], dt.bfloat16, kind="ExternalInput").ap()
    s_in = nc.dram_tensor("s_in", [NBLK, 128, SCH * 128], dt.float8e4, kind="ExternalInput").ap()
    rmt = nc.dram_tensor("rmt", [TCAP, NCHAN], dt.float8e4, kind="ExternalInput").ap()
    part_out = nc.dram_tensor("part_out", [1, NCHAN], dt.float32, kind="ExternalOutput").ap()

    ybd = nc.dram_tensor("ybd", [48, 128], dt.bfloat16).ap()
    if debug_taps == 'brd8':
        dbg_brd8 = nc.dram_tensor("dbg_brd8", [128, WCOLS], dt.float32, kind="ExternalOutput").ap()
    elif debug_taps == 'fold':
        dbg_fold = nc.dram_tensor("dbg_fold", [128, NBLK // 2 * 32], dt.float32, kind="ExternalOutput").ap()

    wout_r = wout.rearrange("(c q) n -> q c n", q=128)   # [128, 36, NWIN]
    rmt_r = rmt.rearrange("(c q) n -> q c n", q=128)     # [128, 20, NCHAN]
    s_r = s_in.rearrange("b q s -> q b s")               # [128, NBLK, SCH*128]

    with tile.TileContext(nc) as tc, ExitStack() as ctx:
        singles = ctx.enter_context(tc.tile_pool(name="singles", bufs=1))

        # small loads on the DVE queue (wout owns the sync queue)
        hbd_sb = singles.tile([128, NPAIR, 2, E2], dt.float8e4)
        nc.scalar.dma_start(hbd_sb[:], hbd.rearrange("q (p h e) -> q p h e", p=NPAIR, h=2))
        means_sb = singles.tile([E2, NWIN], dt.bfloat16)
        nc.scalar.dma_start(means_sb[:], means[:])
        a_sb = singles.tile([128, 9 * 128], dt.bfloat16)
        nc.scalar.dma_start(a_sb[:], a_pm[:])
        dxw_sb = singles.tile([128, VCOLS], dt.bfloat16)
        nc.scalar.dma_start(dxw_sb[:], dxw[:])
        s_sb = singles.tile([128, NBLK, SCH * 128], dt.float8e4)
        nc.scalar.dma_start(s_sb[:], s_r)
        ones_sb = singles.tile([E2, 1], dt.float32)
        nc.vector.memset(ones_sb[:], 0.0)
        nc.vector.memset(ones_sb[:E, :], 1.0)
        y_bf = singles.tile([1, 48 * 128], dt.bfloat16)
        nc.vector.memset(y_bf[:, NWIN:], 0.0)
        fold_t = singles.tile([128, NBLK // 2, 2, 16], dt.float8e4)

        # rmt pair tiles: DMAs queued on sync AFTER wout (issued up front,
        # FIFO keeps them behind the wout stream)
        rpool = ctx.enter_context(tc.tile_pool(name="rt", bufs=10))

        # ---------- phase 1: DoubleRow einsum -> y window ----------
        with tc.tile_pool(name="wt", bufs=8) as wpool, \
             tc.tile_pool(name="ps_o", bufs=2, space="PSUM") as po, \
             tc.tile_pool(name="ex", bufs=4) as epool, \
             tc.tile_pool(name="ps_y", bufs=2, space="PSUM") as py:
            for s, supw in enumerate(SUP):
                sc0 = sum(SUP[:s])
                psum_o = po.tile([E2, supw], dt.float32)
                for p in range(NPAIR):
                    wt = wpool.tile([128, 2, supw], dt.float8e4)
                    nc.sync.dma_start(
                        wt[:], wout_r[:, 2 * p:2 * p + 2, sc0:sc0 + supw])
                    nsub = (supw + 511) // 512
                    for js in range(nsub):
                        j0 = js * 512
                        jw = min(512, supw - j0)
                        if stop_after == 0:
                            if p == NPAIR - 1:
                                nc.tensor.matmul(
                                    psum_o[:, j0:j0 + jw],
                                    lhsT=hbd_sb[:, p],
                                    rhs=wt[:, :, j0:j0 + jw],
                                    start=True, stop=True,
                                    perf_mode=DR)
                        else:
                            nc.tensor.matmul(
                                psum_o[:, j0:j0 + jw],
                                lhsT=hbd_sb[:, p],
                                rhs=wt[:, :, j0:j0 + jw],
                                start=(p == 0), stop=(p == NPAIR - 1),
                                perf_mode=DR)
                for js in range((supw + 511) // 512):
                    j0 = js * 512
                    jw = min(512, supw - j0)
                    col = sc0 + j0
                    t2 = epool.tile([E2, jw], dt.float32)
                    nc.vector.tensor_add(t2[:], psum_o[:, j0:j0 + jw],
                                         means_sb[:, col:col + jw])
                    ex = epool.tile([E2, jw], dt.float32)
                    nc.scalar.activation(ex[:], t2[:],
                                         mybir.ActivationFunctionType.Exp,
                                         scale=LN10 / WSCALE)
                    psy = py.tile([1, jw], dt.float32)
                    nc.tensor.matmul(psy[:], lhsT=ones_sb[:], rhs=ex[:],
                                     start=True, stop=True)
                    nc.vector.tensor_copy(y_bf[:, col:col + jw], psy[:])
                # y slice -> DRAM (row-major [48,128] view), scalar queue
                dw = supw if s < len(SUP) - 1 else 48 * 128 - sc0
                nc.scalar.dma_start(
                    ybd.rearrange("a b -> (a b)")[sc0:sc0 + dw],
                    y_bf[:, sc0:sc0 + dw])

        # ---------- phase 2: transpose window + banded conv ----------
        if stop_after == 1:
            out_sb = singles.tile([1, NCHAN], dt.float32)
            nc.vector.memset(out_sb[:], 0.0)
            nc.vector.tensor_copy(out_sb[:, 0:1], y_bf[:, 0:1])
            nc.scalar.dma_start(part_out[:], out_sb[:])
            nc.compile()
            return nc
        v_sb = singles.tile([128, VCOLS], dt.bfloat16)
        nc.vector.memset(v_sb[:, 0:LPAD], 0.0)
        nc.vector.memset(v_sb[:, LPAD + TCOLS:], 0.0)
        nc.sync.dma_start(v_sb[:, LPAD:LPAD + TCOLS], ybd[:], transpose=True)
        with tc.tile_pool(name="cv", bufs=1) as cvp, \
             tc.tile_pool(name="ps_c", bufs=1, space="PSUM") as pc:
            u_sb = cvp.tile([128, VCOLS], dt.bfloat16)
            nc.vector.tensor_mul(u_sb[:], v_sb[:], dxw_sb[:])
            ps_num = pc.tile([128, WCOLS], dt.float32)
            ps_den = pc.tile([128, WCOLS], dt.float32)
            for jc in range(9):
                nc.tensor.matmul(ps_num[:], lhsT=a_sb[:, jc * 128:(jc + 1) * 128],
                                 rhs=u_sb[:, UOFF + jc:UOFF + jc + WCOLS],
                                 start=(jc == 0), stop=(jc == 8))
            for jc in range(9):
                nc.tensor.matmul(ps_den[:], lhsT=a_sb[:, jc * 128:(jc + 1) * 128],
                                 rhs=dxw_sb[:, UOFF + jc:UOFF + jc + WCOLS],
                                 start=(jc == 0), stop=(jc == 8))
            dene = cvp.tile([128, WCOLS], dt.float32)
            nc.vector.tensor_scalar_add(dene[:], ps_den[:], 1e-30)
            rec = cvp.tile([128, WCOLS], dt.float32)
            nc.vector.reciprocal(rec[:], dene[:])
            brd = cvp.tile([128, WCOLS], dt.float32)
            nc.vector.tensor_mul(brd[:], ps_num[:], rec[:])
            # cast to fp8 (broad is already beta-normalized via the exp bias)
            brd8 = singles.tile([128, WCOLS], dt.float8e4)
            nc.scalar.activation(brd8[:], brd[:],
                                 mybir.ActivationFunctionType.Copy, scale=1.0)
            if debug_taps == 'brd8':
                db8 = cvp.tile([128, WCOLS], dt.float32)
                nc.vector.tensor_copy(db8[:], brd8[:])
                nc.scalar.dma_start(dbg_brd8[:], db8[:])

            # ---------- phase 3: S interp matmul -> fold ----------
            with tc.tile_pool(name="ps_f", bufs=2, space="PSUM") as pf:
                for b in range(NBLK):
                    ps_fold = pf.tile([128, 1], dt.float32)
                    for jc in range(SCH):
                        nc.tensor.matmul(
                            ps_fold[:],
                            lhsT=s_sb[:, b, jc * 128:(jc + 1) * 128],
                            rhs=brd8[:, C0 + CB[b] + jc:C0 + CB[b] + jc + 1],
                            start=(jc == 0), stop=(jc == SCH - 1))
                    nc.vector.tensor_copy(fold_t[:, b // 2, b % 2, 0:1],
                                          ps_fold[:])

        if debug_taps == 'fold':
            dfl = singles.tile([128, NBLK // 2 * 32], dt.float32)
            nc.vector.tensor_copy(
                dfl[:], fold_t.rearrange("q a b c -> q (a b c)"))
            nc.scalar.dma_start(dbg_fold[:], dfl[:])

        # ---------- phase 4: response matvec (DoubleRow) ----------
        out_sb = singles.tile([1, NCHAN], dt.float32)
        with tc.tile_pool(name="ps_m", bufs=1, space="PSUM") as pm:
            pso = pm.tile([1, NCHAN], dt.float32)
            for pb in range(NBLK // 2):
                rt = rpool.tile([128, 2, NCHAN], dt.float8e4)
                nc.sync.dma_start(rt[:], rmt_r[:, 2 * pb:2 * pb + 2, :])
                for nb in range(8):
                    nc.tensor.matmul(
                        pso[:, nb * 512:(nb + 1) * 512],
                        lhsT=fold_t[:, pb, :, 0:1],
                        rhs=rt[:, :, nb * 512:(nb + 1) * 512],
                        start=(pb == 0), stop=(pb == NBLK // 2 - 1),
                        perf_mode=DR)
            nc.vector.tensor_copy(out_sb[:], pso[:])
        nc.scalar.dma_start(part_out[:], out_sb[:])

    nc.compile()
    return nc


def _get_nc():
    global _NC
    if _NC is None:
        _NC = _build_nc()
    return _NC


# ----------------------------------------------------------------------
# host-side planning
# ----------------------------------------------------------------------
def _plan(inputs):
    temp = np.asarray(inputs['temp'], f32).reshape(-1)[0]
    ab = np.asarray(inputs['abundances'], f32).copy().reshape(-1)
    ab[:5] = 1.0
    logz = np.asarray(inputs['logz'], f32).reshape(-1)[0]
    norm = np.asarray(inputs['norm'], f32).reshape(-1)[0]
    vel = np.asarray(inputs['velocity'], f32).reshape(-1)[0]
    W1 = np.asarray(inputs['W1'], f32); b1 = np.asarray(inputs['b1'], f32)
    W2 = np.asarray(inputs['W2'], f32); b2 = np.asarray(inputs['b2'], f32)
    W3 = np.asarray(inputs['W3'], f32); b3 = np.asarray(inputs['b3'], f32)
    Wout = np.asarray(inputs['Wout'], f32); bout = np.asarray(inputs['bout'], f32)
    scales = np.asarray(inputs['scales'], f32)
    means = np.asarray(inputs['means'], f32)
    x = np.asarray(inputs['x'], f32); dx = np.asarray(inputs['dx'], f32)
    new_x = np.asarray(inputs['new_x'], f32)
    resp = np.asarray(inputs['spec_resp'], f32)
    rm = np.asarray(inputs['rm'], f32)

    h = np.tanh(temp * W1[:, 0, :] + b1)
    h = np.tanh(np.einsum('eh,ehk->ek', h, W2) + b2)
    h = np.tanh(np.einsum('eh,ehk->ek', h, W3) + b3)

    z = 10.0 ** np.float64(logz)
    stdev = max(np.float64(vel), 1e-30) * 1000.0 / C_LIGHT
    nrm = np.float64(norm) * (1e22 / LD) ** 2

    ecent = x.astype(np.float64) / (1.0 + z)
    nx = new_x.astype(np.float64)
    j = np.clip(np.searchsorted(ecent, nx) - 1, 0, N - 2)
    wgt = np.clip((nx - ecent[j]) / (ecent[j + 1] - ecent[j]), 0.0, 1.0)

    # shared Toeplitz Gaussian (log grid -> shift invariant)
    d_step = 3.0 / (N - 1)
    k = np.arange(-BAND, BAND + 1, dtype=np.float64)
    D = 10.0 ** (k * d_step) - 1.0
    with np.errstate(under='ignore'):
        g = np.exp(-0.5 * (D / stdev) ** 2)
    jj = np.arange(9 * 128)[:, None]
    mm = np.arange(128)[None, :]
    off = jj - VSH - mm
    valid = (off >= -BAND) & (off <= BAND)
    A = np.where(valid, g[np.clip(off + BAND, 0, 2 * BAND)], 0.0)
    A_pm = np.ascontiguousarray(
        A.reshape(9, 128, 128).transpose(1, 0, 2)).reshape(128, 9 * 128).astype(bf16)

    # block-diagonal FFN head, padded to 4608 rows / 32 elements, fp8
    Hbd = np.zeros((KPAD, E2), f32)
    for e in range(E):
        Hbd[e * HID:(e + 1) * HID, e] = h[e]
    hbd_pm = np.ascontiguousarray(
        Hbd.reshape(NPAIR, 2, 128, E2).transpose(2, 0, 1, 3)).reshape(
        128, NPAIR * 2 * E2)
    hbd_pm = np.clip(hbd_pm, -240, 240).astype(f8e4)

    lgab = np.log10(np.maximum(ab.astype(np.float64), 1e-300))
    lgab = np.maximum(lgab, -80.0)

    # per-bin magnitude estimate of y for fp8 scale planning
    with np.errstate(under='ignore'):
        yhat_full = None  # computed per window below

    fold_f = resp.astype(np.float64) * nrm * (1.0 + z) ** 2
    maskR = nx >= ecent[-1]
    nI = int(np.count_nonzero(~maskR))
    CBa = np.asarray(CB, np.int64)

    cores = []
    WS2 = np.float64(WSCALE)
    for c in range(NCORES):
        t0 = c * TCH
        cnt = min(max(nI - t0, 0), TCH)
        if cnt > 0:
            jlo = int(j[t0]); jhi = int(j[t0 + cnt - 1]) + 1
        else:
            jlo = N - 2; jhi = N - 1
        A_c = jlo - 301   # halo below; qwin(t0)-128*C0 = 45
        A_c = max(0, min(A_c, N - NWIN))
        assert jlo - A_c >= 301, (c, jlo, A_c)
        assert jhi + 301 <= A_c + NWIN, (c, jhi, A_c)

        # Wout window, scales folded, x64, fp8, padded to 4608 rows
        blk = (Wout[:, :, A_c:A_c + NWIN]
               * (scales[:, None, A_c:A_c + NWIN] * WSCALE))
        wpad = np.zeros((KPAD, NWIN), f8e4)
        wpad[:KR] = np.clip(blk.reshape(KR, NWIN), -240, 240).astype(f8e4)

        # fp8 scale planning: yhat from means alone (o*s ignored, bounded)
        with np.errstate(under='ignore'):
            yhat = (ab[:, None] * 10.0 ** (
                means[:, A_c:A_c + NWIN].astype(np.float64)
                + bout[:, A_c:A_c + NWIN].astype(np.float64)
                * scales[:, A_c:A_c + NWIN])).sum(axis=0)
        beta = 16.0 / max(float(yhat.max()), 1e-300)
        lbeta = math.log10(beta)

        # exp bias; +log10(beta) makes the device y (and broad) beta-scaled
        mbuf = np.full((E2, NWIN), -80.0 * WSCALE, f32)
        mbuf[:E] = ((means[:, A_c:A_c + NWIN].astype(np.float64)
                     + bout[:, A_c:A_c + NWIN].astype(np.float64)
                     * scales[:, A_c:A_c + NWIN]
                     + lgab[:, None] + lbeta) * WS2).astype(f32)
        mbuf_bf = mbuf.astype(bf16)

        # dx with window halo (v col layout: bin = A_c - VSH + 128*col + q)
        qv = np.arange(VCOLS * 128, dtype=np.int64)
        gi = A_c + qv - 128 * LPAD
        okm = (gi >= 0) & (gi < N)
        dxv = np.where(okm, dx[np.clip(gi, 0, N - 1)], 0.0).astype(np.float64)
        dxw_pm = np.ascontiguousarray(dxv.reshape(VCOLS, 128).T).astype(bf16)

        # S matrix (block-sparse interp incl. fold factors), fp8-normalized
        qwin = j - A_c
        S = np.zeros((NBLK, SCH, 128, 128), np.float64)
        rmt_b = np.zeros((TCAP, NCHAN), f8e4)
        gamma = 2.0 ** 14
        if cnt > 0:
            te = t0 + cnt
            r = qwin[t0:te] - 128 * C0
            assert r.min() >= 0 and r.max() + 1 < SCH * 128 + 128 * CBa[-1] + 1 \
                and r.max() + 1 < WCOLS * 128, (r.min(), r.max())
            slot = np.arange(cnt)
            bq = slot // 128
            sp = slot % 128
            pos = r - 128 * CBa[bq]
            assert pos.min() >= 0, pos.min()
            assert (pos + 1).max() < SCH * 128, (pos + 1).max()
            wl = (1.0 - wgt[t0:te]) * fold_f[t0:te]
            wr = wgt[t0:te] * fold_f[t0:te]
            np.add.at(S, (bq, pos // 128, pos % 128, sp), wl)
            p2 = pos + 1
            np.add.at(S, (bq, p2 // 128, p2 % 128, sp), wr)
            rmt_b[:cnt] = np.clip(rm[:, t0:te].T * gamma, 0, 240).astype(f8e4)

        # estimated fold magnitude -> alpha scaling for S
        # exact per-channel estimate: interp of yhat
        if cnt > 0:
            ye = yhat[np.clip(qwin[t0:te], 0, NWIN - 1)]
            fest = fold_f[t0:te] * ye
            Mf = max(float(fest.max()), 1e-300)
        else:
            Mf = 1e-300
        alpha = 16.0 / Mf
        # device: fold_dev = (alpha/beta * S) @ (beta * broad)
        S_dev = S * (alpha / beta)

        vc = (nI - 1) // TCH if nI > 0 else 0
        if maskR.any() and c == vc:
            vs = (nI - 1) % TCH + 1 if nI > 0 else 0
            qR = (N - 1) - A_c
            assert 128 * C0 <= qR < WCOLS * 128, qR
            vb = min(vs // 128, NBLK - 1)
            vpos = (qR - 128 * C0) - 128 * int(CBa[vb])
            assert 0 <= vpos < SCH * 128, vpos
            S_dev[vb, vpos // 128, vpos % 128, vs % 128] = alpha / beta
            rv = (rm[:, maskR].astype(np.float64) @ fold_f[maskR])
            rmt_b[vs] = np.clip(rv * gamma, 0, 240).astype(f8e4)

        s_dev8 = np.clip(
            np.ascontiguousarray(S_dev.transpose(0, 2, 1, 3)).reshape(
                NBLK, 128, SCH * 128), -240, 240).astype(f8e4)

        cores.append(dict(wout=wpad, means=mbuf_bf, dxw=dxw_pm,
                          s_in=s_dev8, rmt=rmt_b, beta=beta,
                          unscale=Mf / (16.0 * gamma)))

    return dict(hbd=hbd_pm, A_pm=A_pm, cores=cores)


def make_in_maps(inputs):
    P = _plan(inputs)
    in_maps = []
    for c in range(NCORES):
        pc = P['cores'][c]
        in_maps.append({
            "wout": pc['wout'],
            "hbd": P['hbd'],
            "means": pc['means'],
            "a_pm": P['A_pm'],
            "dxw": pc['dxw'],
            "s_in": pc['s_in'],
            "rmt": pc['rmt'],
        })
    return in_maps, [pc['unscale'] for pc in P['cores']], \
        [pc['beta'] for pc in P['cores']]


def kernel(**inputs) -> np.ndarray:
    nc = _get_nc()
    in_maps, unscales, betas = make_in_maps(inputs)
    res = run_bass_kernel_spmd(nc, in_maps, list(range(NCORES)))
    acc = np.zeros(NCHAN, np.float64)
    for c in range(NCORES):
        acc += (np.asarray(res.results[c]["part_out"], f32).reshape(-1)
                .astype(np.float64) * unscales[c])
    return acc.astype(f32)


# revision 15
# speedup vs baseline: 1.1148x; 1.1148x over previous
"""Trainium2 Bass kernel for nn_CombinedModel_80315888435653.

Pipeline (per forward):
  1. per-element FFN emulators  ->  summed spectrum y      [N=50125]
  2. banded Gaussian velocity broadening (halfwidth 300)   [N]
  3. redshift + linear rebin onto instrument grid          [N_NEW=20000]
  4. ARF scale + response matmul rm @ folded               [N_CHAN=4096]

Distribution strategy (8 NeuronCores, SPMD, full inputs in / full out):
  * Channel-sharded, fully independent cores - NO collective.  Core c
    owns instrument channels [2500c, 2500c+2500); it computes the summed
    spectrum y only on the 6016-bin energy window its channels touch
    (incl. the +-300 broadening halo), broadens it, interpolates, and
    multiplies its 2500-column slice of the response matrix.  Host sums
    the 8 partial [4096] outputs.  Windows of neighbouring cores overlap
    by ~600 bins (~10% duplicated Wout traffic) - far cheaper than an
    AllGather + replicated broadening.
  * The dominant HBM stream (Wout, 27 MB/core) and the response slice
    (10.5 MB/core) are shipped as fp8e4 (scales folded in, x64 exponent
    trick compensated in the exp bias) and consumed with DoubleRow
    matmuls (2 fp8 K-chunks per instruction).
  * Broadening is 18 accumulated Toeplitz matmuls in bf16 (the Gaussian
    collapses to a 601-tap shift-invariant kernel on the log grid).
  * Interp is a per-core static block-sparse fp8 matrix; everything is
    scale-normalized on host so fp8 dynamic range is safe, and the one
    global scalar is applied to the final host-side sum.
"""
import math
import os
import sys
from contextlib import ExitStack

import numpy as np

for _p in ('/opt/trn_rl_repo', '/root/.axon_site/_ro/trn_rl_repo'):
    if os.path.isdir(_p) and _p not in sys.path:
        sys.path.insert(0, _p)

import ml_dtypes  # noqa: E402
import concourse.bass as bass  # noqa: E402
import concourse.tile as tile  # noqa: E402
from concourse import bacc, mybir  # noqa: E402
from concourse.bass_utils import run_bass_kernel_spmd  # noqa: E402

bf16 = ml_dtypes.bfloat16
f8e4 = ml_dtypes.float8_e4m3
f32 = np.float32

# ---- problem constants ----
C_LIGHT = 299792458.0
N = 50125
BAND = 300
E = 30
HID = 150
NNEW = 20000
NCHAN = 4096
LD = 3.086e24

# ---- plan constants ----
NCORES = 8
KR = E * HID                 # 4500 contraction rows
NPAIR = 18                   # 36 K-chunks as 18 DoubleRow pairs
KPAD = NPAIR * 256           # 4608
E2 = 32                      # element rows padded to 32 (DoubleRow stride)
NWIN = 5888                  # per-core einsum window (46 * 128)
WCOLS = NWIN // 128          # 46
SUP = [1024, 1024, 1024, 1024, 1024, 768]   # supercol widths (sum NWIN)
VSH = 384                    # 3-col left halo shift in v
TCOLS = 48                   # transpose-padded col count (%16)
LPAD = 16                    # left pad cols (16-aligned XBAR transpose dst)
VCOLS = LPAD + TCOLS + 9     # 73
UOFF = LPAD - 3              # conv rhs col offset (A_pm embeds VSH=384=3 cols)
C0 = 2                       # S window starts at broad col 2 (bin off 256)
NBLK = 20                    # fold blocks (128 channels each)
SCH = 4                      # S K-chunks per block
TCH = NNEW // NCORES         # 2500 channels per core
TCAP = NBLK * 128            # 2560
LN10 = float(np.log(10.0))
WSCALE = 64.0                # fp8 Wout prescale, compensated in exp bias

RHO = (math.log10(50.0) - math.log10(0.15)) / (NNEW - 1) / (3.0 / (N - 1))
CB = [max(0, math.floor((128 * b * RHO - 4) / 128.0)) for b in range(NBLK)]


# ----------------------------------------------------------------------
# device program (built & compiled once per process)
# ----------------------------------------------------------------------
_NC = None


def _build_nc(sim_single=False, stop_after=None, debug_taps=False):
    dt = mybir.dt
    DR = mybir.MatmulPerfMode.DoubleRow
    nc = bacc.Bacc("TRN2", target_bir_lowering=False, debug=False,
                   num_devices=1 if sim_single else NCORES)

    wout = nc.dram_tensor("wout", [KPAD, NWIN], dt.float8e4, kind="ExternalInput").ap()
    wout_f = wout.rearrange("a b -> (a b)")
    hbd = nc.dram_tensor("hbd", [128, NPAIR * 2 * E2], dt.float8e4, kind="ExternalInput").ap()
    means = nc.dram_tensor("means", [E2, NWIN], dt.bfloat16, kind="ExternalInput").ap()
    a_pm = nc.dram_tensor("a_pm", [128, 9 * 128], dt.bfloat16, kind="ExternalInput").ap()
    dxw = nc.dram_tensor("dxw", [128, VCOLS], dt.bfloat16, kind="ExternalInput").ap()
    s_in = nc.dram_tensor("s_in", [128, NBLK * SCH * 128], dt.float8e4, kind="ExternalInput").ap()
    rmt = nc.dram_tensor("rmt", [TCAP, NCHAN], dt.float8e4, kind="ExternalInput").ap()
    part_out = nc.dram_tensor("part_out", [1, NCHAN], dt.float32, kind="ExternalOutput").ap()

    ybd = nc.dram_tensor("ybd", [48, 128], dt.bfloat16).ap()
    if debug_taps == 'brd8':
        dbg_brd8 = nc.dram_tensor("dbg_brd8", [128, WCOLS], dt.float32, kind="ExternalOutput").ap()
    elif debug_taps == 'fold':
        dbg_fold = nc.dram_tensor("dbg_fold", [128, NBLK // 2 * 32], dt.float32, kind="ExternalOutput").ap()

    rmt_f = rmt.rearrange("a b -> (a b)")

    with tile.TileContext(nc) as tc, ExitStack() as ctx:
        singles = ctx.enter_context(tc.tile_pool(name="singles", bufs=1))

        # small loads on the DVE queue (wout owns the sync queue)
        hbd_sb = singles.tile([128, NPAIR, 2, E2], dt.float8e4)
        nc.scalar.dma_start(hbd_sb[:], hbd.rearrange("q (p h e) -> q p h e", p=NPAIR, h=2))
        means_sb = singles.tile([E2, NWIN], dt.bfloat16)
        nc.scalar.dma_start(means_sb[:], means[:])
        a_sb = singles.tile([128, 9 * 128], dt.bfloat16)
        nc.scalar.dma_start(a_sb[:], a_pm[:])
        dxw_sb = singles.tile([128, VCOLS], dt.bfloat16)
        nc.scalar.dma_start(dxw_sb[:], dxw[:])
        s_sb = singles.tile([128, NBLK, SCH * 128], dt.float8e4)
        nc.scalar.dma_start(s_sb[:], s_in.rearrange("q (b s) -> q b s", b=NBLK))
        ones_sb = singles.tile([E2, 1], dt.float32)
        nc.vector.memset(ones_sb[:], 0.0)
        nc.vector.memset(ones_sb[:E, :], 1.0)
        y_bf = singles.tile([1, 48 * 128], dt.bfloat16)
        nc.vector.memset(y_bf[:, NWIN:], 0.0)
        fold_t = singles.tile([128, NBLK // 2, 2, 16], dt.float8e4)

        # rmt pair tiles: DMAs queued on sync AFTER wout (issued up front,
        # FIFO keeps them behind the wout stream)
        rpool = ctx.enter_context(tc.tile_pool(name="rt", bufs=10))

        # ---------- phase 1: DoubleRow einsum -> y window ----------
        with tc.tile_pool(name="wt", bufs=8) as wpool, \
             tc.tile_pool(name="ps_o", bufs=2, space="PSUM") as po, \
             tc.tile_pool(name="ex", bufs=4) as epool, \
             tc.tile_pool(name="ps_y", bufs=2, space="PSUM") as py:
            for s, supw in enumerate(SUP):
                sc0 = sum(SUP[:s])
                psum_o = po.tile([E2, supw], dt.float32)
                blk0 = NPAIR * 256 * sc0
                for p in range(NPAIR):
                    wt = wpool.tile([128, 2, supw], dt.float8e4)
                    off = blk0 + p * 256 * supw
                    nc.sync.dma_start(
                        wt[:], wout_f[off:off + 256 * supw].rearrange(
                            "(q h n) -> q h n", q=128, h=2))
                    nsub = (supw + 511) // 512
                    for js in range(nsub):
                        j0 = js * 512
                        jw = min(512, supw - j0)
                        if stop_after == 0:
                            if p == NPAIR - 1:
                                nc.tensor.matmul(
                                    psum_o[:, j0:j0 + jw],
                                    lhsT=hbd_sb[:, p],
                                    rhs=wt[:, :, j0:j0 + jw],
                                    start=True, stop=True,
                                    perf_mode=DR)
                        else:
                            nc.tensor.matmul(
                                psum_o[:, j0:j0 + jw],
                                lhsT=hbd_sb[:, p],
                                rhs=wt[:, :, j0:j0 + jw],
                                start=(p == 0), stop=(p == NPAIR - 1),
                                perf_mode=DR)
                for js in range((supw + 511) // 512):
                    j0 = js * 512
                    jw = min(512, supw - j0)
                    col = sc0 + j0
                    t2 = epool.tile([E2, jw], dt.float32)
                    nc.vector.tensor_add(t2[:], psum_o[:, j0:j0 + jw],
                                         means_sb[:, col:col + jw])
                    ex = epool.tile([E2, jw], dt.float32)
                    nc.scalar.activation(ex[:], t2[:],
                                         mybir.ActivationFunctionType.Exp,
                                         scale=LN10 / WSCALE)
                    psy = py.tile([1, jw], dt.float32)
                    nc.tensor.matmul(psy[:], lhsT=ones_sb[:], rhs=ex[:],
                                     start=True, stop=True)
                    nc.vector.tensor_copy(y_bf[:, col:col + jw], psy[:])
                # y slice -> DRAM (row-major [48,128] view), scalar queue
                dw = supw if s < len(SUP) - 1 else 48 * 128 - sc0
                nc.scalar.dma_start(
                    ybd.rearrange("a b -> (a b)")[sc0:sc0 + dw],
                    y_bf[:, sc0:sc0 + dw])

        # ---------- phase 2: transpose window + banded conv ----------
        if stop_after == 1:
            out_sb = singles.tile([1, NCHAN], dt.float32)
            nc.vector.memset(out_sb[:], 0.0)
            nc.vector.tensor_copy(out_sb[:, 0:1], y_bf[:, 0:1])
            nc.scalar.dma_start(part_out[:], out_sb[:])
            nc.compile()
            return nc
        v_sb = singles.tile([128, VCOLS], dt.bfloat16)
        nc.vector.memset(v_sb[:, 0:LPAD], 0.0)
        nc.vector.memset(v_sb[:, LPAD + TCOLS:], 0.0)
        nc.sync.dma_start(v_sb[:, LPAD:LPAD + TCOLS], ybd[:], transpose=True)
        with tc.tile_pool(name="cv", bufs=1) as cvp, \
             tc.tile_pool(name="ps_c", bufs=1, space="PSUM") as pc:
            u_sb = cvp.tile([128, VCOLS], dt.bfloat16)
            nc.vector.tensor_mul(u_sb[:], v_sb[:], dxw_sb[:])
            ps_num = pc.tile([128, WCOLS], dt.float32)
            ps_den = pc.tile([128, WCOLS], dt.float32)
            for jc in range(9):
                nc.tensor.matmul(ps_num[:], lhsT=a_sb[:, jc * 128:(jc + 1) * 128],
                                 rhs=u_sb[:, UOFF + jc:UOFF + jc + WCOLS],
                                 start=(jc == 0), stop=(jc == 8))
            for jc in range(9):
                nc.tensor.matmul(ps_den[:], lhsT=a_sb[:, jc * 128:(jc + 1) * 128],
                                 rhs=dxw_sb[:, UOFF + jc:UOFF + jc + WCOLS],
                                 start=(jc == 0), stop=(jc == 8))
            dene = cvp.tile([128, WCOLS], dt.float32)
            nc.vector.tensor_scalar_add(dene[:], ps_den[:], 1e-30)
            rec = cvp.tile([128, WCOLS], dt.float32)
            nc.vector.reciprocal(rec[:], dene[:])
            brd = cvp.tile([128, WCOLS], dt.float32)
            nc.vector.tensor_mul(brd[:], ps_num[:], rec[:])
            # cast to fp8 (broad is already beta-normalized via the exp bias)
            brd8 = singles.tile([128, WCOLS], dt.float8e4)
            nc.scalar.activation(brd8[:], brd[:],
                                 mybir.ActivationFunctionType.Copy, scale=1.0)
            if debug_taps == 'brd8':
                db8 = cvp.tile([128, WCOLS], dt.float32)
                nc.vector.tensor_copy(db8[:], brd8[:])
                nc.scalar.dma_start(dbg_brd8[:], db8[:])

            # ---------- phase 3: S interp matmul -> fold ----------
            with tc.tile_pool(name="ps_f", bufs=2, space="PSUM") as pf:
                for b in range(NBLK):
                    ps_fold = pf.tile([128, 1], dt.float32)
                    for jc in range(SCH):
                        nc.tensor.matmul(
                            ps_fold[:],
                            lhsT=s_sb[:, b, jc * 128:(jc + 1) * 128],
                            rhs=brd8[:, C0 + CB[b] + jc:C0 + CB[b] + jc + 1],
                            start=(jc == 0), stop=(jc == SCH - 1))
                    nc.vector.tensor_copy(fold_t[:, b // 2, b % 2, 0:1],
                                          ps_fold[:])

        if debug_taps == 'fold':
            dfl = singles.tile([128, NBLK // 2 * 32], dt.float32)
            nc.vector.tensor_copy(
                dfl[:], fold_t.rearrange("q a b c -> q (a b c)"))
            nc.scalar.dma_start(dbg_fold[:], dfl[:])

        # ---------- phase 4: response matvec (DoubleRow) ----------
        out_sb = singles.tile([1, NCHAN], dt.float32)
        with tc.tile_pool(name="ps_m", bufs=1, space="PSUM") as pm:
            pso = pm.tile([1, NCHAN], dt.float32)
            for pb in range(NBLK // 2):
                rt = rpool.tile([128, 2, NCHAN], dt.float8e4)
                roff = pb * 256 * NCHAN
                nc.sync.dma_start(
                    rt[:], rmt_f[roff:roff + 256 * NCHAN].rearrange(
                        "(q h n) -> q h n", q=128, h=2))
                for nb in range(8):
                    nc.tensor.matmul(
                        pso[:, nb * 512:(nb + 1) * 512],
                        lhsT=fold_t[:, pb, :, 0:1],
                        rhs=rt[:, :, nb * 512:(nb + 1) * 512],
                        start=(pb == 0), stop=(pb == NBLK // 2 - 1),
                        perf_mode=DR)
            nc.vector.tensor_copy(out_sb[:], pso[:])
        nc.scalar.dma_start(part_out[:], out_sb[:])

    nc.compile()
    return nc


def _get_nc():
    global _NC
    if _NC is None:
        _NC = _build_nc()
    return _NC


# ----------------------------------------------------------------------
# host-side planning
# ----------------------------------------------------------------------
def _plan(inputs):
    temp = np.asarray(inputs['temp'], f32).reshape(-1)[0]
    ab = np.asarray(inputs['abundances'], f32).copy().reshape(-1)
    ab[:5] = 1.0
    logz = np.asarray(inputs['logz'], f32).reshape(-1)[0]
    norm = np.asarray(inputs['norm'], f32).reshape(-1)[0]
    vel = np.asarray(inputs['velocity'], f32).reshape(-1)[0]
    W1 = np.asarray(inputs['W1'], f32); b1 = np.asarray(inputs['b1'], f32)
    W2 = np.asarray(inputs['W2'], f32); b2 = np.asarray(inputs['b2'], f32)
    W3 = np.asarray(inputs['W3'], f32); b3 = np.asarray(inputs['b3'], f32)
    Wout = np.asarray(inputs['Wout'], f32); bout = np.asarray(inputs['bout'], f32)
    scales = np.asarray(inputs['scales'], f32)
    means = np.asarray(inputs['means'], f32)
    x = np.asarray(inputs['x'], f32); dx = np.asarray(inputs['dx'], f32)
    new_x = np.asarray(inputs['new_x'], f32)
    resp = np.asarray(inputs['spec_resp'], f32)
    rm = np.asarray(inputs['rm'], f32)

    h = np.tanh(temp * W1[:, 0, :] + b1)
    h = np.tanh(np.einsum('eh,ehk->ek', h, W2) + b2)
    h = np.tanh(np.einsum('eh,ehk->ek', h, W3) + b3)

    z = 10.0 ** np.float64(logz)
    stdev = max(np.float64(vel), 1e-30) * 1000.0 / C_LIGHT
    nrm = np.float64(norm) * (1e22 / LD) ** 2

    ecent = x.astype(np.float64) / (1.0 + z)
    nx = new_x.astype(np.float64)
    j = np.clip(np.searchsorted(ecent, nx) - 1, 0, N - 2)
    wgt = np.clip((nx - ecent[j]) / (ecent[j + 1] - ecent[j]), 0.0, 1.0)

    # shared Toeplitz Gaussian (log grid -> shift invariant)
    d_step = 3.0 / (N - 1)
    k = np.arange(-BAND, BAND + 1, dtype=np.float64)
    D = 10.0 ** (k * d_step) - 1.0
    with np.errstate(under='ignore'):
        g = np.exp(-0.5 * (D / stdev) ** 2)
    jj = np.arange(9 * 128)[:, None]
    mm = np.arange(128)[None, :]
    off = jj - VSH - mm
    valid = (off >= -BAND) & (off <= BAND)
    A = np.where(valid, g[np.clip(off + BAND, 0, 2 * BAND)], 0.0)
    A_pm = np.ascontiguousarray(
        A.reshape(9, 128, 128).transpose(1, 0, 2)).reshape(128, 9 * 128).astype(bf16)

    # block-diagonal FFN head, padded to 4608 rows / 32 elements, fp8
    Hbd = np.zeros((KPAD, E2), f32)
    for e in range(E):
        Hbd[e * HID:(e + 1) * HID, e] = h[e]
    hbd_pm = np.ascontiguousarray(
        Hbd.reshape(NPAIR, 2, 128, E2).transpose(2, 0, 1, 3)).reshape(
        128, NPAIR * 2 * E2)
    hbd_pm = np.clip(hbd_pm, -240, 240).astype(f8e4)

    lgab = np.log10(np.maximum(ab.astype(np.float64), 1e-300))
    lgab = np.maximum(lgab, -80.0)

    # per-bin magnitude estimate of y for fp8 scale planning
    with np.errstate(under='ignore'):
        yhat_full = None  # computed per window below

    fold_f = resp.astype(np.float64) * nrm * (1.0 + z) ** 2
    maskR = nx >= ecent[-1]
    nI = int(np.count_nonzero(~maskR))
    CBa = np.asarray(CB, np.int64)

    cores = []
    WS2 = np.float64(WSCALE)
    for c in range(NCORES):
        t0 = c * TCH
        cnt = min(max(nI - t0, 0), TCH)
        if cnt > 0:
            jlo = int(j[t0]); jhi = int(j[t0 + cnt - 1]) + 1
        else:
            jlo = N - 2; jhi = N - 1
        A_c = jlo - 301   # halo below; qwin(t0)-128*C0 = 45
        A_c = max(0, min(A_c, N - NWIN))
        assert jlo - A_c >= 301, (c, jlo, A_c)
        assert jhi + 301 <= A_c + NWIN, (c, jhi, A_c)

        # Wout window, scales folded, x64, fp8, padded to 4608 rows
        blk = (Wout[:, :, A_c:A_c + NWIN]
               * (scales[:, None, A_c:A_c + NWIN] * WSCALE))
        wpad = np.zeros((KPAD, NWIN), f8e4)
        wpad[:KR] = np.clip(blk.reshape(KR, NWIN), -240, 240).astype(f8e4)
        w3 = wpad.reshape(NPAIR, 2, 128, NWIN)
        segs = []
        sc0 = 0
        for supw in SUP:
            segs.append(np.ascontiguousarray(
                w3[:, :, :, sc0:sc0 + supw].transpose(0, 2, 1, 3)).reshape(-1))
            sc0 += supw
        wq = np.concatenate(segs).reshape(KPAD, NWIN)

        # fp8 scale planning: yhat from means alone (o*s ignored, bounded)
        with np.errstate(under='ignore'):
            yhat = (ab[:, None] * 10.0 ** (
                means[:, A_c:A_c + NWIN].astype(np.float64)
                + bout[:, A_c:A_c + NWIN].astype(np.float64)
                * scales[:, A_c:A_c + NWIN])).sum(axis=0)
        beta = 16.0 / max(float(yhat.max()), 1e-300)
        lbeta = math.log10(beta)

        # exp bias; +log10(beta) makes the device y (and broad) beta-scaled
        mbuf = np.full((E2, NWIN), -80.0 * WSCALE, f32)
        mbuf[:E] = ((means[:, A_c:A_c + NWIN].astype(np.float64)
                     + bout[:, A_c:A_c + NWIN].astype(np.float64)
                     * scales[:, A_c:A_c + NWIN]
                     + lgab[:, None] + lbeta) * WS2).astype(f32)
        mbuf_bf = mbuf.astype(bf16)

        # dx with window halo (v col layout: bin = A_c - VSH + 128*col + q)
        qv = np.arange(VCOLS * 128, dtype=np.int64)
        gi = A_c + qv - 128 * LPAD
        okm = (gi >= 0) & (gi < N)
        dxv = np.where(okm, dx[np.clip(gi, 0, N - 1)], 0.0).astype(np.float64)
        dxw_pm = np.ascontiguousarray(dxv.reshape(VCOLS, 128).T).astype(bf16)

        # S matrix (block-sparse interp incl. fold factors), fp8-normalized
        qwin = j - A_c
        S = np.zeros((NBLK, SCH, 128, 128), np.float64)
        rmt_b = np.zeros((TCAP, NCHAN), f8e4)
        gamma = 2.0 ** 14
        if cnt > 0:
            te = t0 + cnt
            r = qwin[t0:te] - 128 * C0
            assert r.min() >= 0 and r.max() + 1 < SCH * 128 + 128 * CBa[-1] + 1 \
                and r.max() + 1 < WCOLS * 128, (r.min(), r.max())
            slot = np.arange(cnt)
            bq = slot // 128
            sp = slot % 128
            pos = r - 128 * CBa[bq]
            assert pos.min() >= 0, pos.min()
            assert (pos + 1).max() < SCH * 128, (pos + 1).max()
            wl = (1.0 - wgt[t0:te]) * fold_f[t0:te]
            wr = wgt[t0:te] * fold_f[t0:te]
            np.add.at(S, (bq, pos // 128, pos % 128, sp), wl)
            p2 = pos + 1
            np.add.at(S, (bq, p2 // 128, p2 % 128, sp), wr)
            rmt_b[:cnt] = np.clip(rm[:, t0:te].T * gamma, 0, 240).astype(f8e4)

        # estimated fold magnitude -> alpha scaling for S
        # exact per-channel estimate: interp of yhat
        if cnt > 0:
            ye = yhat[np.clip(qwin[t0:te], 0, NWIN - 1)]
            fest = fold_f[t0:te] * ye
            Mf = max(float(fest.max()), 1e-300)
        else:
            Mf = 1e-300
        alpha = 16.0 / Mf
        # device: fold_dev = (alpha/beta * S) @ (beta * broad)
        S_dev = S * (alpha / beta)

        vc = (nI - 1) // TCH if nI > 0 else 0
        if maskR.any() and c == vc:
            vs = (nI - 1) % TCH + 1 if nI > 0 else 0
            qR = (N - 1) - A_c
            assert 128 * C0 <= qR < WCOLS * 128, qR
            vb = min(vs // 128, NBLK - 1)
            vpos = (qR - 128 * C0) - 128 * int(CBa[vb])
            assert 0 <= vpos < SCH * 128, vpos
            S_dev[vb, vpos // 128, vpos % 128, vs % 128] = alpha / beta
            rv = (rm[:, maskR].astype(np.float64) @ fold_f[maskR])
            rmt_b[vs] = np.clip(rv * gamma, 0, 240).astype(f8e4)

        s_dev8 = np.clip(
            np.ascontiguousarray(S_dev.transpose(0, 2, 1, 3)).reshape(
                NBLK, 128, SCH * 128), -240, 240).astype(f8e4)
        s_pm = np.ascontiguousarray(s_dev8.transpose(1, 0, 2)).reshape(
            128, NBLK * SCH * 128)
        rmt_q = np.ascontiguousarray(
            rmt_b.reshape(NBLK // 2, 2, 128, NCHAN).transpose(0, 2, 1, 3)
        ).reshape(TCAP, NCHAN)

        cores.append(dict(wout=wq, means=mbuf_bf, dxw=dxw_pm,
                          s_in=s_pm, rmt=rmt_q, beta=beta,
                          unscale=Mf / (16.0 * gamma)))

    return dict(hbd=hbd_pm, A_pm=A_pm, cores=cores)


def make_in_maps(inputs):
    P = _plan(inputs)
    in_maps = []
    for c in range(NCORES):
        pc = P['cores'][c]
        in_maps.append({
            "wout": pc['wout'],
            "hbd": P['hbd'],
            "means": pc['means'],
            "a_pm": P['A_pm'],
            "dxw": pc['dxw'],
            "s_in": pc['s_in'],
            "rmt": pc['rmt'],
        })
    return in_maps, [pc['unscale'] for pc in P['cores']], \
        [pc['beta'] for pc in P['cores']]


def kernel(**inputs) -> np.ndarray:
    nc = _get_nc()
    in_maps, unscales, betas = make_in_maps(inputs)
    res = run_bass_kernel_spmd(nc, in_maps, list(range(NCORES)))
    acc = np.zeros(NCHAN, np.float64)
    for c in range(NCORES):
        acc += (np.asarray(res.results[c]["part_out"], f32).reshape(-1)
                .astype(np.float64) * unscales[c])
    return acc.astype(f32)
